# revision 67
# baseline (speedup 1.0000x reference)
"""Trainium2 Bass kernel for cross-attention (b=2, n=m=2048, dim=1024, 16 heads x 64)
with QK-RMSNorm and rotate-half RoPE (float positions), distributed over 8 NeuronCores.

Sharding: core c handles batch b = c//4 and head group hg = c%4 (4 heads each).
Wq/Wkv are column-sharded by head, Wo row-sharded; each core emits a partial
[2048, 1024] output which the host sums over the 4 cores of each batch (the
row-parallel all-reduce done at unshard time).

Device dataflow per core (all matmuls bf16, softmax exp split ACT/DVE):
  big input loads stream on both DMA queues (~150GB/s each) with the KV pair
  for contraction-chunk kc alternating queues, so the KV projection starts
  ~10us in and is fed at the aggregate rate; the tiny trig inputs ride to
  partition 0 and are replicated on-chip (gpsimd partition_broadcast) --
  128-way-replicating or element-strided DMAs are descriptor storms
  Q  = tgt @ Wq, KV = src @ Wkv  (PE; 2-chunk groups with the contraction
                               interleaved so the PE sees 16-matmul bursts and
                               ramps its clock; drains + x^2 stats on ACT)
  per-4-chunk-group tails, pipelined under the projections:
    rsq via DVE fast-inverse-sqrt (2 Newton steps; K folds the *8, Q folds the
    1/sqrt(hd) score scale), RoPE via w-folded sin/cos tables (DVE TT ops; trig
    from ACT Sin with Cody-Waite range reduction; cos = sin(x+pi/2)),
    then PE identity-matmul transposes (psum bf16) + ACT/DVE copies into
    per-group [hd, t] tiles (the DMA-xbar transpose path is descriptor-bound
    at ~10us per tile and serializes into the attention phase)
  S^T = K_hat^T.T @ Q_hat^T   (row-tiled head pairs, K_c=64, concurrent on PE)
  P^T = exp(S^T)              (psum->sbuf bf16 [128, 1024] ops; ~3/4 on the
                               ScalarE exp table, ~1/4 on DVE via a one-op
                               Schraudolph: bf16_bits = trunc(x*128*log2e +
                               128*(127-c)+0.5) written as int16 and bitcast,
                               so the two engines stream exp concurrently; the
                               whole attention is software-pipelined one
                               (head-pair, q-block) iteration ahead, with AV
                               emitted before the next iteration's QK so the
                               PE always has ready work at boundaries)
  O^T/denom = [V|1].T @ P^T   (PE, M=65 via 128-wide padded weight reads: AV and
                               softmax denominator in one accumulation)
  normalize O^T (DVE fast reciprocal + gpsimd partition_broadcast; both heads'
                 reciprocals are emitted before the oT multiplies so the
                 in-order DVE queue never parks on a broadcast wait)
  out = O^T.T @ Wo            (PE; emitted inside the next q-block's chunk loop
                               so the exp stream never stalls; bf16 output,
                               upcast and summed across cores on the host)
"""

import math
import os

import numpy as np

B, N, DIM, H, HD = 2, 2048, 1024, 16, 64
NCORES = 8
HPC = 4  # heads per core
QD = HPC * HD  # 256
P = 128
NT = N // P  # 16 token chunks
KC = DIM // P  # 8 contraction chunks
QB = 4  # q blocks of 512
QW = N // QB  # 512
ROPE_THETA = 10000.0
EPS = float(np.finfo(np.float32).eps)

# Schraudolph exp in bf16 bit space: bits = trunc(x*SCHRA_A + SCHRA_B), viewed
# as bf16. +0.5 folds the trunc->round correction; c=0.043677 centers the
# +-3% approximation error.
SCHRA_A = 128.0 * math.log2(math.e)
SCHRA_B = 128.0 * (127.0 - 0.043677) + 0.5

# which score chunks each (head-pair, q-block) iteration offloads to DVE:
# hp0 iterations also carry the Wo-drain copies, so they offload fewer.
OFF_HP0 = (5, 11)
OFF_HP1 = (1, 3, 5, 7, 9, 11)
OFF_HS = (1, 4)  # head-start chunks offloaded to DVE (during the Q projection)

_CACHE = {}
LAST_RESULTS = None


def _build_trig(nc, tc, pool, scr, pos_sb, w_sb, invf_sb, consts, tag):
    """sin/cos tables with RMSNorm-weight w folded in. Returns (cw1, cw2, sw1, sw2),
    each [P, NT, 32] bf16: cw1=cos*w[0:32], cw2=cos*w[32:64], sw1=sin*w[0:32],
    sw2=sin*w[32:64]."""
    import concourse.bass as bass
    from concourse import mybir

    f32 = mybir.dt.float32
    bf16 = mybir.dt.bfloat16
    AF = mybir.ActivationFunctionType
    ALU = mybir.AluOpType
    INV2PI, MAGIC, C1, C2, C3 = consts

    ang = scr.tile([P, NT * 32], f32, tag="ang", name=f"ang{tag}")
    nc.vector.tensor_tensor(
        ang.rearrange("p (t j) -> p t j", j=32),
        pos_sb[:, :, None].to_broadcast([P, NT, 32]),
        invf_sb[:, None, :].to_broadcast([P, NT, 32]),
        ALU.mult,
    )
    # round(ang / 2pi) via magic-number rounding
    kf = scr.tile([P, NT * 32], f32, tag="kf", name=f"kf{tag}")
    nc.vector.tensor_scalar(kf, ang, float(INV2PI), float(MAGIC), ALU.mult, ALU.add)
    nc.vector.tensor_scalar(kf, kf, float(MAGIC), None, ALU.subtract)
    angr = scr.tile([P, NT * 32], f32, tag="angr", name=f"angr{tag}")
    nc.vector.cody_waite_cascade(
        out=angr, x=ang, k=kf, c1=float(C1), c2=float(C2), c3=float(C3)
    )
    # cos argument: wrap(angr + pi/2) into [-pi, pi]
    nc.vector.add_range_wrap(
        out=kf, in_=angr, shift=math.pi / 2, bound=math.pi, period=2 * math.pi
    )
    sint = scr.tile([P, NT, 32], f32, tag="sin", name=f"sin{tag}")
    cost = scr.tile([P, NT, 32], f32, tag="cos", name=f"cos{tag}")
    nc.scalar.activation(sint.rearrange("p t j -> p (t j)"), angr, AF.Sin)
    nc.scalar.activation(cost.rearrange("p t j -> p (t j)"), kf, AF.Sin)

    tabs = []
    for name, trig, wlo in (
        ("cw1", cost, True),
        ("cw2", cost, False),
        ("sw1", sint, True),
        ("sw2", sint, False),
    ):
        t = pool.tile([P, NT, 32], bf16, tag=f"{name}{tag}")
        wsl = w_sb[:, 0:32] if wlo else w_sb[:, 32:64]
        nc.vector.tensor_tensor(
            t, trig, wsl[:, None, :].to_broadcast([P, NT, 32]), ALU.mult
        )
        tabs.append(t)
    return tabs


def _tail_group(nc, g, ss, rsq, c, xnat, tabs, xb, xhat, xTg, nm, pools, eps64):
    """After projection chunks 4g..4g+3: rsqrt (DVE fast-inverse-sqrt, 2 NR steps),
    normalize+rope the group, bounce to DRAM, transposed-load into xTg[half][g]."""
    from concourse import mybir

    f32 = mybir.dt.float32
    i32 = mybir.dt.int32
    bf16 = mybir.dt.bfloat16
    ALU = mybir.AluOpType
    acts, ascr = pools[0], pools[1]
    cw1, cw2, sw1, sw2 = tabs
    MAGIC = 0x5F3759DF
    G = 4
    gs = slice(4 * g, 4 * g + 4)

    xg = ascr.tile([P, G, HPC], f32, tag="rsx", name="rsx")
    yg = ascr.tile([P, G, HPC], f32, tag="rsy", name="rsy")
    tg = ascr.tile([P, G, HPC], f32, tag="rst", name="rst")
    nc.vector.tensor_scalar_add(xg, ss[:, gs], eps64)
    nc.vector.tensor_scalar(
        tg.bitcast(i32), xg.bitcast(i32), 1, None, ALU.arith_shift_right
    )
    nc.vector.tensor_scalar(
        yg.bitcast(i32), tg.bitcast(i32), -1, MAGIC, ALU.mult, ALU.add
    )
    nc.vector.tensor_tensor(tg, yg, yg, ALU.mult)
    nc.vector.tensor_tensor(tg, tg, xg, ALU.mult)
    nc.vector.tensor_scalar(tg, tg, -0.5, 1.5, ALU.mult, ALU.add)
    nc.vector.tensor_tensor(yg, yg, tg, ALU.mult)
    nc.vector.tensor_tensor(tg, yg, yg, ALU.mult)
    nc.vector.tensor_tensor(tg, tg, xg, ALU.mult)
    nc.vector.tensor_scalar(tg, tg, -0.5 * c, 1.5 * c, ALU.mult, ALU.add)
    nc.vector.tensor_tensor(rsq[:, gs], yg, tg, ALU.mult)

    x4 = xnat.rearrange("p t (h d) -> p t h d", h=HPC)
    nc.vector.tensor_tensor(
        xb[:, gs],
        x4[:, gs],
        rsq[:, gs, :, None].to_broadcast([P, G, HPC, HD]),
        ALU.mult,
    )
    x1 = xb[:, gs, :, 0:32]
    x2 = xb[:, gs, :, 32:64]
    sh4 = [P, G, HPC, 32]

    def bc(t):
        return t[:, gs, None, :].to_broadcast(sh4)

    a = ascr.tile(sh4, bf16, tag="ra", name="ra")
    b = ascr.tile(sh4, bf16, tag="rb", name="rb")
    nc.vector.tensor_tensor(a, x1, bc(cw1), ALU.mult)
    nc.vector.tensor_tensor(b, x2, bc(sw2), ALU.mult)
    nc.vector.tensor_sub(xhat[:, gs, :, 0, :], a, b)
    a2 = ascr.tile(sh4, bf16, tag="ra", name="ra2")
    b2 = ascr.tile(sh4, bf16, tag="rb", name="rb2")
    nc.vector.tensor_tensor(a2, x1, bc(sw1), ALU.mult)
    nc.vector.tensor_tensor(b2, x2, bc(cw2), ALU.mult)
    nc.vector.tensor_add(xhat[:, gs, :, 1, :], a2, b2)

    # transpose on the PE (identity-matmul) instead of a DRAM bounce + DMA
    # transpose: the DMA xbar path is descriptor-bound (~10us per 128x512
    # tile) and serializes the queues into the attention phase
    trpsum, ident = pools[2], pools[3]
    for half in range(2):
        for tc_ in range(G):
            src = xhat[:, 4 * g + tc_, 2 * half : 2 * half + 2].rearrange(
                "p h two j -> p (h two j)"
            )
            tp = trpsum.tile([P, P], bf16, tag="trp", name=f"trp{half}{tc_}")
            nc.tensor.transpose(tp, src, ident)
            dst = xTg[half][g][:, tc_ * P : (tc_ + 1) * P]
            if (half * G + tc_) % 2 == 0:
                nc.scalar.copy(dst, tp)
            else:
                nc.vector.tensor_copy(dst, tp)

def _build():
    import concourse.bass as bass
    import concourse.tile as tile
    from concourse import bacc, mybir

    f32 = mybir.dt.float32
    i16 = mybir.dt.int16
    bf16 = mybir.dt.bfloat16
    AF = mybir.ActivationFunctionType
    ALU = mybir.AluOpType

    nc = bacc.Bacc(
        "TRN2", target_bir_lowering=False, debug=False, num_devices=NCORES
    )

    tgt_t = nc.dram_tensor("tgt_t", [DIM, N], bf16, kind="ExternalInput").ap()
    src_t = nc.dram_tensor("src_t", [DIM, N], bf16, kind="ExternalInput").ap()
    wq_d = nc.dram_tensor("wq", [DIM, QD], bf16, kind="ExternalInput").ap()
    wkv_d = nc.dram_tensor("wkv", [DIM, 2 * QD], bf16, kind="ExternalInput").ap()
    wo_d = nc.dram_tensor("wo", [QD, DIM], bf16, kind="ExternalInput").ap()
    tpos = nc.dram_tensor("tpos", [P, NT], f32, kind="ExternalInput").ap()
    spos = nc.dram_tensor("spos", [P, NT], f32, kind="ExternalInput").ap()
    qw_d = nc.dram_tensor("qw", [HD], f32, kind="ExternalInput").ap()
    kw_d = nc.dram_tensor("kw", [HD], f32, kind="ExternalInput").ap()
    out_d = nc.dram_tensor("out", [N, DIM], bf16, kind="ExternalOutput").ap()

    invf_np = np.float32(ROPE_THETA) ** (
        -np.arange(0, HD, 2, dtype=np.float32) / np.float32(HD)
    )
    invf_dram = nc.inline_tensor(invf_np.astype(np.float32), "invf").ap()

    TWO_PI = 2 * math.pi
    C1 = np.float32(6.28125)
    C2 = np.float32(TWO_PI - float(C1))
    C3 = np.float32(TWO_PI - float(C1) - float(C2))
    MAGIC = np.float32(1.5 * 2**23)
    INV2PI = np.float32(1.0 / TWO_PI)
    consts = (INV2PI, MAGIC, C1, C2, C3)

    def bcast_ap(src, parts):
        return bass.AP(tensor=src.tensor, offset=src.offset, ap=[[0, parts]] + src.ap)

    DEBUG = bool(int(os.environ.get("KERNEL_DEBUG", "0")))
    dbg_done = set()

    def dbg(name, ap):
        if not DEBUG or name in dbg_done:
            return
        dbg_done.add(name)
        t = nc.dram_tensor(f"d_{name}", list(ap.shape), ap.dtype, kind="ExternalOutput").ap()
        nc.sync.dma_start(out=t, in_=ap)

    with tile.TileContext(nc) as tc:
        with (
            tc.tile_pool(name="persist", bufs=1) as persist,
        ):
            # ---- issue every big input DMA up front, split over two queues:
            # gpsimd feeds the KV projection (src/wkv) plus the small
            # broadcast loads the trig build needs; sync takes tgt/wq (needed
            # ~40us in) and later the tail bounce/transposes.
            qw_sb = persist.tile([P, HD], f32, tag="qw")
            kw_sb = persist.tile([P, HD], f32, tag="kw")
            invf_sb = persist.tile([P, 32], f32, tag="invf")
            spos_sb = persist.tile([P, NT], f32, tag="sposs")
            tpos_sb = persist.tile([P, NT], f32, tag="tposs")
            qw_p0 = persist.tile([1, HD], f32, tag="qwp0")
            kw_p0 = persist.tile([1, HD], f32, tag="kwp0")
            invf_p0 = persist.tile([1, 32], f32, tag="invfp0")

            _acts_cm = tc.tile_pool(name="acts", bufs=1)
            acts = _acts_cm.__enter__()
            xs_bf = []
            xt_bf = []
            wkv_bf = []
            wq_bf = []
            for kc in range(KC):
                wkv_bf.append(
                    acts.tile([P, 2 * QD], bf16, tag=f"wkv{kc}", name=f"wkv{kc}")
                )
                xs_bf.append(acts.tile([P, N], bf16, tag=f"xs{kc}", name=f"xs{kc}"))
                wq_bf.append(acts.tile([P, QD], bf16, tag=f"wq{kc}", name=f"wq{kc}"))
                xt_bf.append(acts.tile([P, N], bf16, tag=f"xt{kc}", name=f"xt{kc}"))
            wo_bf = persist.tile([P, 2, DIM], bf16, tag="wo")

            # DMA queue layout (two queues, ~180GB/s each). The KV projection
            # pair kc streams alternately so the accumulation is fed at the
            # aggregate rate; the trig-table inputs ride to partition 0 (cheap
            # single-row DMAs; the 128-way replication happens on the idle
            # GpSimd engine, not the DMA fabric); Q inputs and Wo follow.
            def kv_pair(kc, eng):
                eng.dma_start(out=wkv_bf[kc], in_=wkv_d[kc * P : (kc + 1) * P, :])
                eng.dma_start(out=xs_bf[kc], in_=src_t[kc * P : (kc + 1) * P, :])

            def q_pair(kc, eng):
                eng.dma_start(out=wq_bf[kc], in_=wq_d[kc * P : (kc + 1) * P, :])
                eng.dma_start(out=xt_bf[kc], in_=tgt_t[kc * P : (kc + 1) * P, :])

            # three DMA queues (sync + gpsimd + the Activation HWDGE ring,
            # idle this early): first pair split so kc=0 lands earliest, the
            # rest round-robin so the KV inputs arrive at ~3x a single
            # queue's ~150GB/s
            nc.scalar.dma_start(out=wkv_bf[0], in_=wkv_d[0:P, :])
            nc.gpsimd.dma_start(out=xs_bf[0], in_=src_t[0:P, :])
            kv_pair(1, nc.sync)
            kv_pair(2, nc.scalar)
            nc.gpsimd.dma_start(out=invf_p0, in_=bcast_ap(invf_dram, 1))
            nc.gpsimd.dma_start(out=kw_p0, in_=bcast_ap(kw_d, 1))
            nc.gpsimd.dma_start(out=qw_p0, in_=bcast_ap(qw_d, 1))
            nc.gpsimd.dma_start(out=spos_sb, in_=spos)
            nc.gpsimd.dma_start(out=tpos_sb, in_=tpos)
            kv_pair(3, nc.gpsimd)
            kv_pair(4, nc.sync)
            kv_pair(5, nc.scalar)
            kv_pair(6, nc.gpsimd)
            kv_pair(7, nc.sync)
            for kc in range(KC):
                q_pair(
                    kc,
                    (nc.sync, nc.scalar, nc.gpsimd)[kc % 3],
                )
            for cc in range(2):
                nc.gpsimd.dma_start(
                    out=wo_bf[:, cc], in_=wo_d[cc * P : (cc + 1) * P, :]
                )
            # replicate the small rows on-chip
            nc.gpsimd.partition_broadcast(invf_sb, invf_p0)
            nc.gpsimd.partition_broadcast(kw_sb, kw_p0)
            nc.gpsimd.partition_broadcast(qw_sb, qw_p0)

            from concourse import masks

            ident = persist.tile([P, P], bf16, tag="ident")
            masks.make_identity(nc, ident[:, :])

            # V (+ softmax-denominator ones) staging; memset early while DVE
            # is idle
            vaug_flat = persist.tile([P, NT * HPC * (HD + 1) + HD - 1], bf16, tag="vaug")
            nc.vector.memset(vaug_flat, 1.0)
            vaug = vaug_flat[:, 0 : NT * HPC * (HD + 1)].rearrange(
                "p (t h d) -> p t h d", h=HPC, d=HD + 1
            )

            # k tables first: the K-projection tails consume them ~15us in
            with tc.tile_pool(name="trigscr", bufs=1) as trigscr:
                tabs_k = _build_trig(
                    nc, tc, persist, trigscr, spos_sb, kw_sb, invf_sb, consts, "k"
                )
                tabs_q = _build_trig(
                    nc, tc, persist, trigscr, tpos_sb, qw_sb, invf_sb, consts, "q"
                )

            kTg = [
                [persist.tile([P, QW], bf16, tag=f"kT{h}_{g}", name=f"kT{h}_{g}") for g in range(QB)]
                for h in range(2)
            ]
            qTg = [
                [persist.tile([P, QW], bf16, tag=f"qT{h}_{g}", name=f"qT{h}_{g}") for g in range(QB)]
                for h in range(2)
            ]
            oT = [persist.tile([P, N], bf16, tag=f"oT{i}", name=f"oT{i}") for i in range(2)]

            _spsum_cm = tc.tile_pool(name="spsum", bufs=2, space="PSUM")
            sp_pool = [_spsum_cm.__enter__()]
            with (
                tc.tile_pool(name="ascr", bufs=2) as ascr,
                tc.tile_pool(name="ppsum", bufs=3, space="PSUM") as ppsum,
                tc.tile_pool(name="trps", bufs=1, space="PSUM") as trpsum,
            ):
                pt0 = persist.tile([P, NT, 2, QW], bf16, tag="pt0")

                # preload the exp table set during idle ACT time (after trig)
                dummy = acts.tile([P, 1], mybir.dt.float32, tag="dummy")
                nc.vector.memset(dummy, 0.0)
                nc.scalar.activation(dummy, dummy, AF.Exp)


                EPS64 = 64.0 * EPS

                def emit_qk_exp(hp, qb, mc, pt, dve):
                    sp = sp_pool[0].tile([P, 2, QW], f32, tag="sstage", name="sp")
                    for i in range(2):
                        pp = slice(i * 64, (i + 1) * 64)
                        nc.tensor.matmul(
                            sp[:, i, :],
                            lhsT=kTg[hp][mc // 4][pp, (mc % 4) * P : (mc % 4 + 1) * P],
                            rhs=qTg[hp][qb][pp, :],
                            start=True,
                            stop=True,
                            tile_position=(i * 64, 0),
                        )
                    if dve:
                        nc.vector.tensor_scalar(
                            pt.bitcast(i16)[:, mc],
                            sp,
                            float(SCHRA_A),
                            float(SCHRA_B),
                            ALU.mult,
                            ALU.add,
                        )
                    else:
                        nc.scalar.activation(pt[:, mc], sp, AF.Exp)

                # ---- KV projection; per-group tail ----
                knat = acts.tile([P, NT, QD], bf16, tag="knat")
                sqK = acts.tile([P, NT, HPC, HD], bf16, tag="sq", name="sqK")
                kss = persist.tile([P, NT, HPC], f32, tag="kss")
                krsq = persist.tile([P, NT, HPC], f32, tag="krsq")
                kxb = acts.tile([P, NT, HPC, HD], bf16, tag="xb", name="kxb")
                khat = acts.tile([P, NT, HPC, 2, 32], bf16, tag="xhat", name="khat")
                # 2-chunk groups with the contraction interleaved: 16
                # back-to-back matmuls per group keep the PE busy long enough
                # to ramp its clock, and absorb DMA-arrival jitter
                for g in range(NT // 2):
                    pss = [
                        ppsum.tile([P, 2 * QD], f32, tag="kvps", name=f"kvps{j}")
                        for j in range(2)
                    ]
                    for kc in range(KC):
                        for j in range(2):
                            nc.tensor.matmul(
                                pss[j],
                                lhsT=xs_bf[kc][:, (2 * g + j) * P : (2 * g + j + 1) * P],
                                rhs=wkv_bf[kc],
                                start=(kc == 0),
                                stop=(kc == KC - 1),
                            )
                    for j in range(2):
                        mc = 2 * g + j
                        nc.scalar.square(
                            sqK[:, mc],
                            pss[j][:, 0:QD].rearrange("p (h d) -> p h d", h=HPC),
                        )
                        nc.scalar.copy(knat[:, mc], pss[j][:, 0:QD])
                        nc.scalar.copy(
                            vaug[:, mc, :, 0:HD],
                            pss[j][:, QD : 2 * QD].rearrange("p (h d) -> p h d", h=HPC),
                        )
                        nc.vector.tensor_reduce(
                            kss[:, mc], sqK[:, mc], axis=mybir.AxisListType.X, op=ALU.add
                        )
                    if g % 2 == 1:
                        # c=8: rsq_k = rsqrt(ms+eps) = 8*rsqrt(sumsq+64eps)
                        _tail_group(
                            nc, g // 2, kss, krsq, 8.0, knat, tabs_k, kxb,
                            khat, kTg, "k", (acts, ascr, trpsum, ident), EPS64,
                        )
                dbg("knat", knat)
                dbg("krsq", krsq)
                dbg("khat", khat)
                # ---- Q projection; per-group tail (rsqrt/rope/transpose) ----
                qnat = acts.tile([P, NT, QD], bf16, tag="qnat")
                sq = acts.tile([P, NT, HPC, HD], bf16, tag="sq")
                qss = persist.tile([P, NT, HPC], f32, tag="qss")
                qrsq = persist.tile([P, NT, HPC], f32, tag="qrsq")
                qxb = acts.tile([P, NT, HPC, HD], bf16, tag="xb")
                qhat = acts.tile([P, NT, HPC, 2, 32], bf16, tag="xhat")
                for g in range(NT // 2):
                    pss = [
                        ppsum.tile([P, 2 * QD], f32, tag="kvps", name=f"qps{j}")
                        for j in range(2)
                    ]
                    for kc in range(KC):
                        for j in range(2):
                            nc.tensor.matmul(
                                pss[j][:, 0:QD],
                                lhsT=xt_bf[kc][:, (2 * g + j) * P : (2 * g + j + 1) * P],
                                rhs=wq_bf[kc],
                                start=(kc == 0),
                                stop=(kc == KC - 1),
                            )
                    for j in range(2):
                        mc = 2 * g + j
                        nc.scalar.square(
                            sq[:, mc],
                            pss[j][:, 0:QD].rearrange("p (h d) -> p h d", h=HPC),
                        )
                        nc.scalar.copy(qnat[:, mc], pss[j][:, 0:QD])
                        nc.vector.tensor_reduce(
                            qss[:, mc], sq[:, mc], axis=mybir.AxisListType.X, op=ALU.add
                        )
                    if g % 2 == 1:
                        # c=1: rsqrt(sumsq+64eps) = rsqrt(ms+eps)/8 folds the
                        # 1/sqrt(hd) score scale into q
                        _tail_group(
                            nc, g // 2, qss, qrsq, 1.0, qnat, tabs_q, qxb,
                            qhat, qTg, "q", (acts, ascr, trpsum, ident), EPS64,
                        )
                    # head-start: stream iteration-0 scores+exp at a 2-group
                    # lag behind the Q projection (the PE transposes deliver
                    # each qTg group ~1us after its tail, so a short lag is
                    # safe); only 4 chunks remain for the trailing emit, which
                    # keeps the ScalarE queue clear at the attention boundary
                    if g >= 2:
                        for j in range(2):
                            amc = 2 * g + j - 4
                            emit_qk_exp(0, 0, amc, pt0, amc in OFF_HS)
                dbg("qnat", qnat)
                dbg("qrsq", qrsq)
                dbg("qhat", qhat)
                for amc in range(12, NT):
                    emit_qk_exp(0, 0, amc, pt0, amc in OFF_HS)

            _acts_cm.__exit__(None, None, None)
            dbg("vaug", vaug)
            dbg("cw1q", tabs_q[0])
            dbg("sw1q", tabs_q[2])
            # ---- attention + output projection ----
            with (
                tc.tile_pool(name="avpsum", bufs=2, space="PSUM") as avpsum,
                tc.tile_pool(name="ptp", bufs=2) as ptp,
                tc.tile_pool(name="dnp", bufs=4) as dnp,
                tc.tile_pool(name="ostage", bufs=4) as ostage,
            ):
                def emit_wo_tc(qb, ti):
                    t0 = qb * QW + ti * P
                    ost = ostage.tile([P, DIM], bf16, tag="ost", name="ost")
                    for od in range(2):
                        wps = avpsum.tile([P, QW], f32, tag=f"av{od}", name="wps")
                        for cc in range(2):
                            nc.tensor.matmul(
                                wps,
                                lhsT=oT[cc][:, t0 : t0 + P],
                                rhs=wo_bf[:, cc, od * 512 : (od + 1) * 512],
                                start=(cc == 0),
                                stop=(cc == 1),
                            )
                        nc.vector.tensor_copy(ost[:, od * 512 : (od + 1) * 512], wps)
                    nc.sync.dma_start(out=out_d[t0 : t0 + P, :], in_=ost)

                pending = []
                its = [(qb, hp) for qb in range(QB) for hp in range(2)]
                pts = {0: pt0}
                for idx, (qb, hp) in enumerate(its):
                    if True:
                        pt = pts[idx]
                        av = [
                            avpsum.tile([P, QW], f32, tag=f"av{i}", name=f"av{i}") for i in range(2)
                        ]
                        for mc in range(NT):
                            # AV first: the start of an iteration is never
                            # gated on the score-psum rotation, so the PE
                            # always has ready work at iteration boundaries
                            for i in range(2):
                                base = (mc * HPC + 2 * hp + i) * (HD + 1)
                                nc.tensor.matmul(
                                    av[i],
                                    lhsT=vaug_flat[:, base : base + P],
                                    rhs=pt[:, mc, i, :],
                                    start=(mc == 0),
                                    stop=(mc == NT - 1),
                                )
                            # the NEXT iteration's scores+exp stream one step
                            # ahead so the exp engines (ACT + DVE) never stall
                            if idx + 1 < len(its):
                                if mc == 0:
                                    pts[idx + 1] = ptp.tile(
                                        [P, NT, 2, QW], bf16, tag="pt", name="pt"
                                    )
                                nqb, nhp = its[idx + 1]
                                off = OFF_HP0 if nhp == 0 else OFF_HP1
                                emit_qk_exp(nhp, nqb, mc, pts[idx + 1], mc in off)
                            if hp == 0 and mc % 4 == 3 and pending:
                                pending.pop(0)()
                            pts.pop(idx - 1, None)
                        if qb == 0 and hp == 0:
                            dbg("pt", pt)
                        # normalize: both heads' den/recip first (DVE), the
                        # partition-broadcasts run on GpSimd underneath, and
                        # the oT multiplies come last so the in-order DVE
                        # queue never parks on a broadcast wait
                        dns = []
                        for i in range(2):
                            den = dnp.tile([1, QW], f32, tag="den", name=f"den{i}")
                            nc.vector.tensor_copy(den, av[i][HD : HD + 1, :])
                            dn = dnp.tile([1, QW], f32, tag="dn", name=f"dn{i}")
                            nc.vector.reciprocal_approx_fast(out=dn, in_=den)
                            dnb = dnp.tile([HD, QW], f32, tag="dnb", name=f"dnb{i}")
                            nc.gpsimd.partition_broadcast(dnb, dn)
                            dns.append(dnb)
                            if qb == 0 and hp == 0 and i == 0:
                                dbg("dn", dn)
                                dbg("dnb", dnb)
                        for i in range(2):
                            nc.vector.tensor_tensor(
                                oT[hp][i * HD : (i + 1) * HD, qb * QW : (qb + 1) * QW],
                                av[i][0:HD, :],
                                dns[i],
                                ALU.mult,
                            )
                    if qb == QB - 1:
                        dbg("oT0", oT[0])
                        dbg("oT1", oT[1])
                    # queue this q block's output projection; emitted inside
                    # the next q block's hp0 chunk loop to keep exp fed
                    if qb != QB - 1:
                        pending = [
                            (lambda q, t: lambda: emit_wo_tc(q, t))(qb, ti)
                            for ti in range(QW // P)
                        ]
                # drain: the last q block's Wo. The cc=0 halves of the first
                # two token tiles run inside the final normalize's PE-idle
                # window (accumulating in retired score-psum tiles), which
                # also keeps the PE clock warm for the rest of the drain.
                wps_pre = []
                for ti in range(2):
                    w = sp_pool[0].tile([P, 2, QW], f32, tag="sstage", name=f"wodr{ti}")
                    t0 = (QB - 1) * QW + ti * P
                    for od in range(2):
                        nc.tensor.matmul(
                            w[:, od, :],
                            lhsT=oT[0][:, t0 : t0 + P],
                            rhs=wo_bf[:, 0, od * 512 : (od + 1) * 512],
                            start=True,
                            stop=False,
                        )
                    wps_pre.append(w)
                for ti in range(2):
                    t0 = (QB - 1) * QW + ti * P
                    w = wps_pre[ti]
                    ost = ostage.tile([P, DIM], bf16, tag="ost", name=f"osd{ti}")
                    for od in range(2):
                        nc.tensor.matmul(
                            w[:, od, :],
                            lhsT=oT[1][:, t0 : t0 + P],
                            rhs=wo_bf[:, 1, od * 512 : (od + 1) * 512],
                            start=False,
                            stop=True,
                        )
                        nc.vector.tensor_copy(
                            ost[:, od * 512 : (od + 1) * 512], w[:, od, :]
                        )
                    nc.sync.dma_start(out=out_d[t0 : t0 + P, :], in_=ost)
                for ti in range(2, QW // P):
                    emit_wo_tc(QB - 1, ti)
            _spsum_cm.__exit__(None, None, None)

    nc.compile()
    return nc


def _get_nc():
    if "nc" not in _CACHE:
        _CACHE["nc"] = _build()
    return _CACHE["nc"]


def _shard(inputs):
    tgt = np.asarray(inputs["tgt"], np.float32)
    src = np.asarray(inputs["src"], np.float32)
    tgt_pos = np.asarray(inputs["tgt_pos"], np.float32)
    src_pos = np.asarray(inputs["src_pos"], np.float32)
    Wq = np.asarray(inputs["Wq"], np.float32)
    Wkv = np.asarray(inputs["Wkv"], np.float32)
    Wo = np.asarray(inputs["Wo"], np.float32)
    qw = np.asarray(inputs["q_norm_w"], np.float32)
    kw = np.asarray(inputs["k_norm_w"], np.float32)

    import ml_dtypes

    bf = ml_dtypes.bfloat16
    in_maps = []
    for c in range(NCORES):
        b, hg = divmod(c, 4)
        cs = slice(hg * QD, (hg + 1) * QD)
        in_maps.append(
            {
                "tgt_t": np.ascontiguousarray(tgt[b].T.astype(bf)),
                "src_t": np.ascontiguousarray(src[b].T.astype(bf)),
                "wq": np.ascontiguousarray(Wq[:, cs].astype(bf)),
                "wkv": np.ascontiguousarray(
                    np.concatenate([Wkv[:, cs], Wkv[:, DIM:][:, cs]], axis=1).astype(bf)
                ),
                "wo": np.ascontiguousarray(Wo[cs, :].astype(bf)),
                "tpos": np.ascontiguousarray(tgt_pos[b].reshape(NT, P).T),
                "spos": np.ascontiguousarray(src_pos[b].reshape(NT, P).T),
                "qw": np.ascontiguousarray(qw),
                "kw": np.ascontiguousarray(kw),
            }
        )
    return in_maps


def _install_ntff_shim():
    """Provide antenv.axon_hooks (missing in this image) so trace=True can
    capture NTFF profiles through libaxon_pjrt.so."""
    import sys
    import types
    import contextlib
    import ctypes

    if "antenv.axon_hooks" in sys.modules:
        return
    so_path = "/opt/axon/libaxon_pjrt.so"
    if not os.path.exists(so_path):
        return
    lib = ctypes.CDLL(so_path)
    if not hasattr(lib, "axon_start_nrt_profile"):
        return
    lib.axon_start_nrt_profile.argtypes = [
        ctypes.POINTER(ctypes.c_int64),
        ctypes.c_size_t,
    ]
    lib.axon_start_nrt_profile.restype = ctypes.c_int64
    lib.axon_stop_nrt_profile.argtypes = [ctypes.c_char_p]
    lib.axon_stop_nrt_profile.restype = ctypes.c_int64

    @contextlib.contextmanager
    def _hook(output_dir, device_ids):
        import jax

        jax.devices()
        if device_ids:
            ids = (ctypes.c_int64 * len(device_ids))(*device_ids)
            rc = lib.axon_start_nrt_profile(ids, len(device_ids))
        else:
            rc = lib.axon_start_nrt_profile(None, 0)
        if rc != 0:
            raise RuntimeError(f"axon_start_nrt_profile rc={rc}")
        try:
            yield
        finally:
            n = lib.axon_stop_nrt_profile(str(output_dir).encode())
            print(f"ntff profile: {n} file(s) written to {output_dir}")

    mod = types.ModuleType("antenv.axon_hooks")
    mod.get_axon_ntff_profile_hook = lambda: _hook
    mod.set_axon_ntff_profile_hook = lambda h: None
    sys.modules["antenv.axon_hooks"] = mod


def kernel(**inputs) -> np.ndarray:
    global LAST_RESULTS
    from concourse.bass_utils import run_bass_kernel_spmd

    nc = _get_nc()
    in_maps = _shard(inputs)
    trace = bool(int(os.environ.get("KERNEL_TRACE", "0")))
    if trace:
        _install_ntff_shim()
    res = run_bass_kernel_spmd(
        nc, in_maps, core_ids=list(range(NCORES)), trace=trace
    )
    LAST_RESULTS = res
    out = np.zeros((B, N, DIM), np.float32)
    for c in range(NCORES):
        out[c // 4] += np.asarray(res.results[c]["out"]).astype(np.float32)
    return out
